# revision 1
# baseline (speedup 1.0000x reference)
"""Trainium2 Bass kernel for nn_MultiHeadAttention (B=4, S=2048, D=1024, H=16).

Sharding: 8 cores = 4 batches x 2 head-groups. Core c handles batch c//2 and
heads (c%2)*8 .. (c%2)*8+7. Each core computes Q/K/V projections for its 512
columns, causal attention for its 8 heads, and a partial output projection
(row-parallel over wo). Host sums the two partials per batch and adds bo.

Per-core kernel layout (fp32 in DRAM, float32r through the PE):
  x^T [D, S] built on-chip via PE transposes, interleaved with the V
  projection so the PE stream stays dense (HAM stays at K=8/8).
  Q^T, K^T per head-pair [128, S] (head dims on partitions -> scores
  contract over head_dim, 2 heads row-packed on the PE). V natural [S, cols]
  with a trailing ones column per head (via augmented weights/bias) so the
  P^T@V matmul also produces softmax row-sums. Attention is computed
  transposed: S^T = K^T.T @ Q^T, exp on ACT, O^T accumulates in PSUM
  [65, 512] chunks (row 64 = sums). Normalization multiplies by reciprocal
  sums broadcast across partitions via DRAM round-trips (reciprocal done on
  a [128, QW/128] reshape to keep DVE time tiny). O^T staged to DRAM;
  final out = O^T.T @ wo per q-tile. Q/K projections for head-pair hp+1 are
  emitted interleaved into hp's attention windows to fill PE gaps.
"""

import numpy as np

import concourse.bass as bass
import concourse.mybir as mybir
import concourse.tile as tile
from concourse import bacc
from concourse.masks import make_upper_triangular

F32 = mybir.dt.float32
F32R = mybir.dt.float32r
P = 128


def build_nc(S=2048, D=1024, HN=8, HD=64, mmdt=F32R):
    """Build the per-core Bass module. HN = local heads, C = HN*HD local cols.

    mmdt: dtype of all matmul operands (F32R or bfloat16). PSUM accumulation
    is fp32 either way; host ships x/weights pre-cast to match."""
    MD = mmdt
    np_md = np.float32 if MD == F32R else mybir.dt.np(MD)
    C = HN * HD
    NT = S // P        # token tiles
    ND = D // P        # d tiles (contraction for projections)
    NM = C // P        # head-pairs (2 heads of HD=64 per 128-partition tile)
    QW = min(512, S)   # O^T psum chunk width (one PSUM bank)
    SW = min(1024, S)  # scores window width (two PSUM banks)
    W5 = min(512, S)   # projection N-chunk width
    WD = min(512, D)   # phase-D N-chunk width
    NCH = S // SW      # score windows per head
    SCALE = 1.0 / float(np.sqrt(HD))
    VW = HD + 1        # V columns per head incl. trailing ones column
    CV = HN * VW       # augmented V cols

    nc = bacc.Bacc("TRN2", target_bir_lowering=False)

    x_d = nc.dram_tensor("x", [S, D], MD if MD == mybir.dt.bfloat16 else F32, kind="ExternalInput")
    wq_d = nc.dram_tensor("wq", [D, C], MD, kind="ExternalInput")
    wk_d = nc.dram_tensor("wk", [D, C], MD, kind="ExternalInput")
    wv_d = nc.dram_tensor("wv", [D, CV], MD, kind="ExternalInput")
    wo_d = nc.dram_tensor("wo", [C, D], MD, kind="ExternalInput")
    bq_d = nc.dram_tensor("bq", [C], F32, kind="ExternalInput")
    bk_d = nc.dram_tensor("bk", [C], F32, kind="ExternalInput")
    bv_d = nc.dram_tensor("bv", [CV], F32, kind="ExternalInput")
    out_d = nc.dram_tensor("out", [S, D], F32, kind="ExternalOutput")

    ident_d = nc.inline_tensor(np.eye(P, dtype=np.float32), name="identity_const").bitcast(F32R)
    xr_d = x_d.bitcast(F32R)

    with tile.TileContext(nc) as tc:
        from contextlib import ExitStack

        with ExitStack() as ctx:
            singles = ctx.enter_context(tc.tile_pool(name="singles", bufs=1))
            ident = singles.tile([P, P], F32R)
            nc.sync.dma_start(ident[:], ident_d[:, :])
            # ut1[k, q] = 1.0 where k <= q else 0 (valid causal region of a
            # diagonal tile in S^T = [k, q] layout).
            ut1 = singles.tile([P, P], F32)
            make_upper_triangular(nc, ut1[:], val=1.0, diag=True)

            bq_sb = singles.tile([P, NM], F32)
            nc.sync.dma_start(bq_sb[:], bq_d.rearrange("(m p) -> p m", p=P))
            bk_sb = singles.tile([P, NM], F32)
            nc.sync.dma_start(bk_sb[:], bk_d.rearrange("(m p) -> p m", p=P))
            # bv broadcast to all partitions via step-0 partition DMA read.
            bv_sb = singles.tile([P, CV], F32)
            nc.sync.dma_start(
                bv_sb[:], bass.AP(tensor=bv_d, offset=0, ap=[[0, P], [1, CV]])
            )

            wqk_pool = ctx.enter_context(tc.tile_pool(name="wqk", bufs=1))
            wq_sb = wqk_pool.tile([P, ND, C], MD)
            wk_sb = wqk_pool.tile([P, ND, C], MD)

            # PSUM pools: two score slots (2 banks each) + 4 O^T slots (1 bank
            # each) = 8 banks. Everything else tag-shares the score slots.
            ps_s0 = ctx.enter_context(tc.tile_pool(name="ps_s0", bufs=1, space="PSUM"))
            ps_s1 = ctx.enter_context(tc.tile_pool(name="ps_s1", bufs=1, space="PSUM"))
            ps_o = ctx.enter_context(tc.tile_pool(name="ps_o", bufs=4, space="PSUM"))

            def s_pool(i):
                return ps_s0 if i % 2 == 0 else ps_s1

            # Mid-kernel-released pools live on the right-side stack, opened
            # in reverse release order (x_nat released first, xT last).
            xT_ctx = ExitStack()
            xT_pool = xT_ctx.enter_context(tc.tile_pool(name="xT", bufs=1, side="right"))
            wv_ctx = ExitStack()
            wv_pool = wv_ctx.enter_context(tc.tile_pool(name="wv", bufs=1, side="right"))
            x_nat_ctx = ExitStack()
            x_nat_pool = x_nat_ctx.enter_context(
                tc.tile_pool(name="x_nat", bufs=3, side="right")
            )

            # ---- Phase A+B: x^T + V projection ---------------------------
            xT = xT_pool.tile([P, ND, S], MD)
            wv_sb = wv_pool.tile([P, ND, CV], MD)
            nc.sync.dma_start(wv_sb[:], wv_d.rearrange("(o p) n -> p o n", p=P))
            v_pool = ctx.enter_context(tc.tile_pool(name="v", bufs=1))
            v_sb = v_pool.tile([P, NT, HN, VW], MD)
            v_pieces = [(lo, min(512, CV - lo)) for lo in range(0, CV, 512)]
            if MD == mybir.dt.bfloat16:
                # 2-byte path: build x^T with the DMA-transpose XBAR, no PE
                # transposes at all.
                TC = min(1024, S)
                for tch in range(S // TC):
                    for d in range(ND):
                        nc.sync.dma_start(
                            xT[:, d, tch * TC:(tch + 1) * TC],
                            x_d[tch * TC:(tch + 1) * TC, d * P:(d + 1) * P],
                            transpose=True,
                        )
                for t in range(NT):
                    psv = s_pool(t).tile([P, CV], F32, tag=f"s{t % 2}", name="psv")
                    for d in range(ND):
                        for lo, w in v_pieces:
                            nc.tensor.matmul(
                                psv[:, lo:lo + w], xT[:, d, t * P:(t + 1) * P],
                                wv_sb[:, d, lo:lo + w],
                                start=(d == 0), stop=(d == ND - 1),
                            )
                    nc.vector.tensor_add(v_sb[:, t], psv[:], bv_sb[:])
            else:
                # fp32 path: per token tile DMA x, 8 PE transposes, then the V
                # matmuls — keeps the PE densely fed from the start.
                for t in range(NT):
                    x_nat = x_nat_pool.tile([P, D], F32R, tag="xn")
                    nc.sync.dma_start(x_nat[:], xr_d[t * P:(t + 1) * P, :])
                    for d in range(ND):
                        psA = s_pool(d).tile([P, P], F32R, tag=f"s{d % 2}", name="psA")
                        nc.tensor.transpose(psA[:], x_nat[:, d * P:(d + 1) * P], ident[:])
                        nc.vector.tensor_copy(xT[:, d, t * P:(t + 1) * P], psA[:])
                    psv = s_pool(t).tile([P, CV], F32, tag=f"s{t % 2}", name="psv")
                    for d in range(ND):
                        for lo, w in v_pieces:
                            nc.tensor.matmul(
                                psv[:, lo:lo + w], xT[:, d, t * P:(t + 1) * P],
                                wv_sb[:, d, lo:lo + w],
                                start=(d == 0), stop=(d == ND - 1),
                            )
                    nc.vector.tensor_add(v_sb[:, t], psv[:], bv_sb[:])
            x_nat_ctx.close()
            wv_ctx.close()
            # Q/K weights arrive after x/wv so the V pipeline starts sooner.
            nc.sync.dma_start(wq_sb[:], wq_d.rearrange("(o p) n -> p o n", p=P))
            nc.sync.dma_start(wk_sb[:], wk_d.rearrange("(o p) n -> p o n", p=P))

            # ---- Phases C (per head-pair) --------------------------------
            NQC = S // QW  # number of O^T chunks per head
            SQF = QW // P  # free size of the [P, SQF] reciprocal reshape
            wo_pool = ctx.enter_context(tc.tile_pool(name="wo", bufs=1))
            wo_sb = wo_pool.tile([P, NM, D], MD)
            nc.sync.dma_start(wo_sb[:], wo_d.rearrange("(f p) n -> p f n", p=P))
            # normalized O^T tiles, kept in SBUF until phase D consumes them;
            # head pairs share a 128-partition tile (odd head moved up by DMA)
            stg_pool = ctx.enter_context(tc.tile_pool(name="stg", bufs=1))
            stg_all = stg_pool.tile([P, NQC, NM, QW], MD)
            qkT_pool = ctx.enter_context(tc.tile_pool(name="qkT", bufs=2))
            pT_pool = ctx.enter_context(tc.tile_pool(name="pT", bufs=4))
            norm_pool = ctx.enter_context(tc.tile_pool(name="norm", bufs=2))
            sums_dram = ctx.enter_context(tc.tile_pool(name="sumsd", bufs=4, space="DRAM"))

            # Row-packing both heads on the PE needs base-partition-64 APs;
            # with bf16 weights that path (FWL) crashed the exec unit, so
            # bf16 uses per-head base-0 tiles and sequential head matmuls.
            PACK = MD == F32R

            def c1_chunks(hp):
                """Q^T/K^T projection for head-pair hp as a list of emitter
                closures (one per psum-group) for interleaved emission."""
                if PACK:
                    qT = qkT_pool.tile([P, S], MD, tag="qT", name="qT")
                    kT = qkT_pool.tile([P, S], MD, tag="kT", name="kT")
                    qTs = [qT[0:HD, :], qT[HD:P, :]]
                    kTs = [kT[0:HD, :], kT[HD:P, :]]
                else:
                    qT0 = qkT_pool.tile([HD, S], MD, tag="qT0", name="qT0")
                    qT1 = qkT_pool.tile([HD, S], MD, tag="qT1", name="qT1")
                    kT0 = qkT_pool.tile([HD, S], MD, tag="kT0", name="kT0")
                    kT1 = qkT_pool.tile([HD, S], MD, tag="kT1", name="kT1")
                    qTs = [qT0[:], qT1[:]]
                    kTs = [kT0[:], kT1[:]]
                chunks = []
                for n in range(S // W5):
                    def emit(n=n):
                        sl = slice(n * W5, (n + 1) * W5)
                        psq = s_pool(n).tile([P, W5], F32, tag=f"s{n % 2}", name="psq")
                        for d in range(ND):
                            nc.tensor.matmul(
                                psq[:], wq_sb[:, d, hp * P:(hp + 1) * P], xT[:, d, sl],
                                start=(d == 0), stop=(d == ND - 1),
                            )
                        if PACK:
                            nc.vector.tensor_scalar_add(
                                qTs[0].tensor[:, sl], psq[:], bq_sb[:, hp:hp + 1])
                        else:
                            nc.vector.tensor_scalar_add(
                                qTs[0][:, sl], psq[0:HD, :], bq_sb[0:HD, hp:hp + 1])
                            nc.vector.tensor_scalar_add(
                                qTs[1][:, sl], psq[HD:P, :], bq_sb[HD:P, hp:hp + 1])
                        psk = s_pool(n + 1).tile([P, W5], F32, tag=f"s{(n + 1) % 2}", name="psk")
                        for d in range(ND):
                            nc.tensor.matmul(
                                psk[:], wk_sb[:, d, hp * P:(hp + 1) * P], xT[:, d, sl],
                                start=(d == 0), stop=(d == ND - 1),
                            )
                        if PACK:
                            nc.vector.tensor_scalar_add(
                                kTs[0].tensor[:, sl], psk[:], bk_sb[:, hp:hp + 1])
                        else:
                            nc.vector.tensor_scalar_add(
                                kTs[0][:, sl], psk[0:HD, :], bk_sb[0:HD, hp:hp + 1])
                            nc.vector.tensor_scalar_add(
                                kTs[1][:, sl], psk[HD:P, :], bk_sb[HD:P, hp:hp + 1])
                    chunks.append(emit)
                return qTs, kTs, chunks

            def drain(chunks, k):
                for _ in range(k):
                    if chunks:
                        chunks.pop(0)()

            ostg_pool = ctx.enter_context(tc.tile_pool(name="ostg", bufs=3))
            d_done = set()

            def emit_d_chunk(qc):
                """Output projection for q-chunk qc: out = sum_h O_h @ wo_h."""
                if qc in d_done:
                    return
                d_done.add(qc)
                for mm in range(QW // P):
                    m = qc * (QW // P) + mm
                    off = mm * P
                    for n in range(D // WD):
                        pso = s_pool(n).tile([P, WD], F32, tag=f"s{n % 2}", name="pso")
                        for f in range(NM):
                            nc.tensor.matmul(
                                pso[:], stg_all[:, qc, f, off:off + P],
                                wo_sb[:, f, n * WD:(n + 1) * WD],
                                start=(f == 0), stop=(f == NM - 1),
                            )
                        ost = ostg_pool.tile([P, WD], F32, tag="ostg", name="ost")
                        nc.vector.tensor_copy(ost[:], pso[:])
                        nc.sync.dma_start(
                            out_d[m * P:(m + 1) * P, n * WD:(n + 1) * WD], ost[:]
                        )

            cur = c1_chunks(0)
            drain(cur[2], len(cur[2]))

            for hp in range(NM):
                qTs, kTs = cur[0], cur[1]
                if hp + 1 < NM:
                    nxt = c1_chunks(hp + 1)
                    nxt_chunks = nxt[2]
                else:
                    nxt = None
                    nxt_chunks = []
                    xT_ctx.close()
                # spread next head-pair's projection chunks across this
                # head-pair's attention iterations
                n_ki_total = 2 * sum(
                    (min((chh + 1) * SW, S)) // P for chh in range(NCH)
                )
                emit_every = max(1, n_ki_total // (len(nxt_chunks) + 1))

                # C2: causal attention, heads processed sequentially with
                # double-buffered score slots (ki alternates the two pools).
                # o_ps[r][qc]: [1+HD, QW] psum accumulators (row HD = sums).
                o_ps = [[None] * NQC for _ in range(2)]
                ki_count = 0
                for rr in range(2):
                    for ch in range(NCH):
                        W0 = ch * SW
                        for ki in range((min((ch + 1) * SW, S)) // P):
                            qlo = max(W0, ki * P)
                            rel = qlo - W0
                            h = 2 * hp + rr
                            s_ps = s_pool(ki).tile([P, SW], F32, tag=f"s{ki % 2}", name="s_ps")
                            # scores^T pieces, split at absolute 512 bounds
                            plo = rel
                            while plo < SW:
                                pw = min(512 - plo % 512, SW - plo)
                                nc.tensor.matmul(
                                    s_ps[:, plo:plo + pw],
                                    kTs[rr][:, ki * P:(ki + 1) * P],
                                    qTs[rr][:, W0 + plo:W0 + plo + pw],
                                    start=True, stop=True,
                                )
                                plo += pw
                            pT = pT_pool.tile([P, SW], MD, tag="pT", name="pT")
                            nc.scalar.activation(
                                pT[:, rel:SW], s_ps[:, rel:SW],
                                mybir.ActivationFunctionType.Exp, scale=SCALE,
                            )
                            if ki * P >= W0:
                                # diagonal tile: zero the strictly-lower part
                                nc.vector.tensor_mul(
                                    pT[:, rel:rel + P], pT[:, rel:rel + P], ut1[:]
                                )
                            # P^T @ V pieces into O^T chunks
                            plo = rel
                            while plo < SW:
                                pw = min(512 - plo % 512, SW - plo)
                                qc = (W0 + plo) // QW
                                lastki = min(qc * QW // P + QW // P - 1, NT - 1)
                                if o_ps[rr][qc] is None:
                                    o_ps[rr][qc] = ps_o.tile(
                                        [VW, QW], F32, tag="o", name="o_ps"
                                    )
                                nc.tensor.matmul(
                                    o_ps[rr][qc][:, (W0 + plo) % QW:(W0 + plo) % QW + pw],
                                    v_sb[:, ki, h, :],
                                    pT[:, plo:plo + pw],
                                    start=(ki == 0), stop=(ki == lastki),
                                )
                                plo += pw
                                if ki == lastki:
                                    # Chunk complete. Copy whole [VW, QW] psum
                                    # to SBUF (frees the o slot fast), then
                                    # normalize rows 0..HD-1 by 1/sums: sums
                                    # (row HD) are reciprocal'd on a [P, SQF]
                                    # reshape and broadcast across partitions,
                                    # both via DRAM round-trips.
                                    ops = o_ps[rr][qc]
                                    oc = norm_pool.tile([VW, QW], F32, tag="oc", name="oc")
                                    nc.vector.tensor_copy(oc[:], ops[:])
                                    sd = sums_dram.tile([1, QW], F32, tag="sd", name="sd")
                                    nc.sync.dma_start(sd[:], oc[HD:VW, :])
                                    sd_ap = sd[:]
                                    sq = norm_pool.tile([P, SQF], F32, tag="sq", name="sq")
                                    nc.sync.dma_start(
                                        sq[:],
                                        bass.AP(tensor=sd_ap.tensor, offset=sd_ap.offset,
                                                ap=[[SQF, P], [1, SQF]]),
                                    )
                                    nc.vector.reciprocal(sq[:], sq[:])
                                    rd = sums_dram.tile([1, QW], F32, tag="rd", name="rd")
                                    rd_ap = rd[:]
                                    nc.sync.dma_start(
                                        bass.AP(tensor=rd_ap.tensor, offset=rd_ap.offset,
                                                ap=[[SQF, P], [1, SQF]]),
                                        sq[:],
                                    )
                                    bc = norm_pool.tile([HD, QW], F32, tag="bc", name="bc")
                                    nc.sync.dma_start(
                                        bc[:],
                                        bass.AP(tensor=rd_ap.tensor, offset=rd_ap.offset,
                                                ap=[[0, HD], [1, QW]]),
                                    )
                                    if rr == 0:
                                        nc.vector.tensor_mul(
                                            stg_all[0:HD, qc, hp, :],
                                            oc[0:HD, :], bc[:],
                                        )
                                    else:
                                        tmpn = norm_pool.tile([HD, QW], MD, tag="tmpn", name="tmpn")
                                        nc.vector.tensor_mul(tmpn[:], oc[0:HD, :], bc[:])
                                        nc.sync.dma_start(
                                            stg_all[HD:P, qc, hp, :], tmpn[:]
                                        )
                                    if hp == NM - 1 and rr == 1:
                                        emit_d_chunk(qc)
                            ki_count += 1
                            if ki_count % emit_every == 0:
                                drain(nxt_chunks, 1)
                drain(nxt_chunks, len(nxt_chunks))
                cur = nxt

            # ---- Phase D remainder: any chunk groups not yet emitted -----
            for qc in range(NQC):
                emit_d_chunk(qc)

    nc.compile()
    return nc


_NC_CACHE = {}

# Matmul operand dtype: bfloat16 (fast path, fp32 PSUM accumulation) or
# float32r (higher precision, ~2x slower PE). Overridable via MM_DT env var.
import os as _os
MM_DT = mybir.dt.bfloat16 if _os.environ.get("MM_DT", "bf16") == "bf16" else F32R


def _get_nc(S, D, HN, HD):
    key = (S, D, HN, HD, MM_DT)
    if key not in _NC_CACHE:
        _NC_CACHE[key] = build_nc(S, D, HN, HD, mmdt=MM_DT)
    return _NC_CACHE[key]


def augment_v(wv_local, bv_local, HN, HD):
    """Append per head a zero weight column with bias 1.0 (softmax-sum col)."""
    D = wv_local.shape[0]
    wv_a = np.zeros((D, HN, HD + 1), dtype=np.float32)
    wv_a[:, :, :HD] = wv_local.reshape(D, HN, HD)
    bv_a = np.ones((HN, HD + 1), dtype=np.float32)
    bv_a[:, :HD] = bv_local.reshape(HN, HD)
    return np.ascontiguousarray(wv_a.reshape(D, -1)), np.ascontiguousarray(bv_a.reshape(-1))


def kernel(**inputs):
    out, _ = run_with_results(inputs)
    return out


def run_with_results(inputs, **spmd_kwargs):
    from concourse.bass_utils import run_bass_kernel_spmd

    x = np.asarray(inputs["x"], dtype=np.float32)
    wq = np.asarray(inputs["wq"], dtype=np.float32)
    bq = np.asarray(inputs["bq"], dtype=np.float32)
    wk = np.asarray(inputs["wk"], dtype=np.float32)
    bk = np.asarray(inputs["bk"], dtype=np.float32)
    wv = np.asarray(inputs["wv"], dtype=np.float32)
    bv = np.asarray(inputs["bv"], dtype=np.float32)
    wo = np.asarray(inputs["wo"], dtype=np.float32)
    bo = np.asarray(inputs["bo"], dtype=np.float32)

    B, S, D = x.shape
    H = 16
    HD = D // H
    G = 2                  # head groups
    HN = H // G            # heads per core
    C = HN * HD
    n_cores = B * G

    nc = _get_nc(S, D, HN, HD)

    np_md = mybir.dt.np(MM_DT) if MM_DT != F32R else np.float32

    in_maps = []
    for c in range(n_cores):
        b, g = c // G, c % G
        sl = slice(g * C, (g + 1) * C)
        in_maps.append({
            "x": np.ascontiguousarray(x[b]).astype(np_md) if MM_DT == mybir.dt.bfloat16 else np.ascontiguousarray(x[b]),
            "wq": np.ascontiguousarray(wq[:, sl]).astype(np_md),
            "wk": np.ascontiguousarray(wk[:, sl]).astype(np_md),
            "wo": np.ascontiguousarray(wo[sl, :]).astype(np_md),
            "bq": np.ascontiguousarray(bq[sl]),
            "bk": np.ascontiguousarray(bk[sl]),
        })
        wv_a, bv_a = augment_v(wv[:, sl], bv[sl], HN, HD)
        in_maps[-1]["wv"] = wv_a.astype(np_md)
        in_maps[-1]["bv"] = bv_a

    res = run_bass_kernel_spmd(nc, in_maps, core_ids=list(range(n_cores)), **spmd_kwargs)
    outs = [m["out"] for m in res.results]
    out = np.stack([sum(outs[b * G + g] for g in range(G)) for b in range(B)])
    out = out + bo[None, None, :]
    return out.astype(np.float32), res



# revision 6
# speedup vs baseline: 1.1488x; 1.1488x over previous
"""Trainium2 Bass kernel for nn_MultiHeadAttention (B=4, S=2048, D=1024, H=16).

Sharding: 8 cores = 4 batches x 2 head-groups. Core c handles batch c//2 and
heads (c%2)*8 .. (c%2)*8+7. Each core computes Q/K/V projections for its 512
columns, causal attention for its 8 heads, and a partial output projection
(row-parallel over wo). Host sums the two partials per batch and adds bo.

Per-core layout (all matmul operands bf16, fp32 PSUM accumulation):
  x^T [D, S] is shipped pre-transposed from the host. Q^T/K^T per head
  [64, S] (head dim on partitions, so scores contract over head_dim).
  V natural [S, 65] per head with a trailing ones column (augmented
  weights/bias) so the P^T@V matmul also produces softmax row-sums.
  Attention is computed transposed: S^T = K^T.T @ Q^T per 128-k-tile into
  [128, 1024] PSUM windows, exp on ACT, O^T accumulates in [65, 512] PSUM
  chunks (row 64 = sums).

Scheduling (the point of this version): the two heads of a head-pair are
interleaved step-by-step (rr alternates per ki) and the PV matmul of step s
is emitted one step late, so the PE never sits behind the exp of the score
tile it just produced. All deferrable projection work (V tiles 8-15, the
next head-pair's Q/K projection, the output projection of finished q-chunks)
is spread evenly into the attention windows as PE filler so the tensor
engine stays busy and the HAM clock gate keeps the PE at 2.4 GHz.
Normalization multiplies O^T rows by 1/sums broadcast via small DRAM
round-trips; the odd (rr=1) head lands in stg rows 0-63 by a direct DVE
write (wo rows are swapped host-side to match) since it finishes last.
"""

import numpy as np

import concourse.bass as bass
import concourse.mybir as mybir
import concourse.tile as tile
from concourse import bacc
from concourse.masks import make_upper_triangular

F32 = mybir.dt.float32
BF16 = mybir.dt.bfloat16
P = 128


def build_nc(S=2048, D=1024, HN=8, HD=64):
    """Per-core Bass module. HN = local heads, C = HN*HD local cols."""
    MD = BF16
    C = HN * HD
    NT = S // P        # token tiles
    ND = D // P        # d tiles (contraction for projections)
    NM = C // P        # head-pairs (2 heads of HD=64 per 128-partition tile)
    QW = 512           # O^T psum chunk width (one PSUM bank)
    SW = 1024          # scores window width (two PSUM banks)
    W5 = 512           # projection N-chunk width
    WD = 512           # phase-D N-chunk width
    NCH = S // SW      # score windows per head
    NQC = S // QW      # O^T chunks per head
    SQF = QW // P      # free size of the [P, SQF] reciprocal reshape
    SCALE = 1.0 / float(np.sqrt(HD))
    VW = HD + 1        # V columns per head incl. trailing ones column
    CV = HN * VW       # augmented V cols

    nc = bacc.Bacc("TRN2", target_bir_lowering=False)

    xT_d = nc.dram_tensor("xT", [D, S], MD, kind="ExternalInput")
    wq_d = nc.dram_tensor("wq", [D, C], MD, kind="ExternalInput")
    wk_d = nc.dram_tensor("wk", [D, C], MD, kind="ExternalInput")
    wv_d = nc.dram_tensor("wv", [D, CV], MD, kind="ExternalInput")
    wo_d = nc.dram_tensor("wo", [C, D], MD, kind="ExternalInput")
    bq_d = nc.dram_tensor("bq", [C], F32, kind="ExternalInput")
    bk_d = nc.dram_tensor("bk", [C], F32, kind="ExternalInput")
    bv_d = nc.dram_tensor("bv", [CV], F32, kind="ExternalInput")
    out_d = nc.dram_tensor("out", [S, D], F32, kind="ExternalOutput")

    with tile.TileContext(nc) as tc:
        from contextlib import ExitStack

        with ExitStack() as ctx:
            singles = ctx.enter_context(tc.tile_pool(name="singles", bufs=1))
            # ut1[k, q] = 1.0 where k <= q else 0 (valid causal region of a
            # diagonal tile in S^T = [k, q] layout).
            ut1 = singles.tile([P, P], MD)
            make_upper_triangular(nc, ut1[:], val=1.0, diag=True)

            bq_sb = singles.tile([P, NM], F32)
            nc.sync.dma_start(bq_sb[:], bq_d.rearrange("(m p) -> p m", p=P))
            bk_sb = singles.tile([P, NM], F32)
            nc.sync.dma_start(bk_sb[:], bk_d.rearrange("(m p) -> p m", p=P))
            # bv broadcast to all partitions via step-0 partition DMA read.
            bv_sb = singles.tile([P, CV], F32)
            nc.sync.dma_start(
                bv_sb[:], bass.AP(tensor=bv_d, offset=0, ap=[[0, P], [1, CV]])
            )

            wqk_pool = ctx.enter_context(tc.tile_pool(name="wqk", bufs=1))
            wq_sb = wqk_pool.tile([P, ND, C], MD)
            wk_sb = wqk_pool.tile([P, ND, C], MD)

            # PSUM: two score slots (2 banks each) + 4 O^T slots (1 bank
            # each) = 8 banks. Projection psums tag-share the score slots.
            ps_s0 = ctx.enter_context(tc.tile_pool(name="ps_s0", bufs=1, space="PSUM"))
            ps_s1 = ctx.enter_context(tc.tile_pool(name="ps_s1", bufs=1, space="PSUM"))
            ps_o = ctx.enter_context(tc.tile_pool(name="ps_o", bufs=4, space="PSUM"))

            def s_pool(i):
                return ps_s0 if i % 2 == 0 else ps_s1

            # Mid-kernel-released pools on the right-side stack: wv first,
            # xT last.
            xT_ctx = ExitStack()
            xT_pool = xT_ctx.enter_context(tc.tile_pool(name="xT", bufs=1, side="right"))
            wv_ctx = ExitStack()
            wv_pool = wv_ctx.enter_context(tc.tile_pool(name="wv", bufs=1, side="right"))

            xT = xT_pool.tile([P, ND, S], MD)
            wv_sb = wv_pool.tile([P, ND, CV], MD)
            nc.sync.dma_start(wv_sb[:], wv_d.rearrange("(o p) n -> p o n", p=P))
            # x^T arrives in token-quarters so V tiles can start early.
            xT_src = xT_d.rearrange("(o p) n -> p o n", p=P)
            TQ = S // 4
            for i in range(4):
                nc.sync.dma_start(
                    xT[:, :, i * TQ:(i + 1) * TQ], xT_src[:, :, i * TQ:(i + 1) * TQ]
                )

            v_pool = ctx.enter_context(tc.tile_pool(name="v", bufs=1))
            v_sb = v_pool.tile([P, NT, HN, VW], MD)
            v_pieces = [(lo, min(512, CV - lo)) for lo in range(0, CV, 512)]

            slot = [0]   # shared psum score-slot parity counter

            def emit_v_tile(t):
                psv = s_pool(slot[0]).tile([P, CV], F32, tag=f"s{slot[0] % 2}", name="psv")
                slot[0] += 1
                for d in range(ND):
                    for lo, w in v_pieces:
                        nc.tensor.matmul(
                            psv[:, lo:lo + w], xT[:, d, t * P:(t + 1) * P],
                            wv_sb[:, d, lo:lo + w],
                            start=(d == 0), stop=(d == ND - 1),
                        )
                nc.vector.tensor_add(v_sb[:, t], psv[:], bv_sb[:])

            # V projection for the first NT//2 token tiles runs up front;
            # the rest interleaves into head-pair 0's first window.
            for t in range(NT // 2):
                emit_v_tile(t)

            # Q/K weights after x/wv so the V pipeline starts sooner.
            nc.sync.dma_start(wq_sb[:], wq_d.rearrange("(o p) n -> p o n", p=P))
            nc.sync.dma_start(wk_sb[:], wk_d.rearrange("(o p) n -> p o n", p=P))

            wo_pool = ctx.enter_context(tc.tile_pool(name="wo", bufs=1))
            wo_sb = wo_pool.tile([P, NM, D], MD)
            nc.sync.dma_start(wo_sb[:], wo_d.rearrange("(f p) n -> p f n", p=P))

            # normalized O^T tiles until phase D; rows 0-63 = odd head (rr=1,
            # DVE-direct), rows 64-127 = even head (rr=0, via DMA). wo rows
            # are swapped host-side to match.
            stg_pool = ctx.enter_context(tc.tile_pool(name="stg", bufs=1))
            stg_all = stg_pool.tile([P, NQC, NM, QW], MD)
            qkT_pool = ctx.enter_context(tc.tile_pool(name="qkT", bufs=2))
            pT_pool = ctx.enter_context(tc.tile_pool(name="pT", bufs=4))
            norm_pool = ctx.enter_context(tc.tile_pool(name="norm", bufs=2))
            sums_dram = ctx.enter_context(tc.tile_pool(name="sumsd", bufs=4, space="DRAM"))
            ostg_pool = ctx.enter_context(tc.tile_pool(name="ostg", bufs=3))

            def c1_emitters(hp, slot):
                """Q^T/K^T projection for head-pair hp: qTs/kTs tiles plus a
                list of 8 emitter closures (one per psum accumulation group)
                for fine-grained interleaved emission."""
                qT0 = qkT_pool.tile([HD, S], MD, tag="qT0", name="qT0")
                qT1 = qkT_pool.tile([HD, S], MD, tag="qT1", name="qT1")
                kT0 = qkT_pool.tile([HD, S], MD, tag="kT0", name="kT0")
                kT1 = qkT_pool.tile([HD, S], MD, tag="kT1", name="kT1")
                qTs = [qT0[:], qT1[:]]
                kTs = [kT0[:], kT1[:]]
                emitters = []
                for n in range(S // W5):
                    for w_sb, b_sb, dsts in ((wq_sb, bq_sb, qTs), (wk_sb, bk_sb, kTs)):
                        def emit(n=n, w_sb=w_sb, b_sb=b_sb, dsts=dsts):
                            sl = slice(n * W5, (n + 1) * W5)
                            psq = s_pool(slot[0]).tile(
                                [P, W5], F32, tag=f"s{slot[0] % 2}", name="psq"
                            )
                            slot[0] += 1
                            for d in range(ND):
                                nc.tensor.matmul(
                                    psq[:], w_sb[:, d, hp * P:(hp + 1) * P],
                                    xT[:, d, sl],
                                    start=(d == 0), stop=(d == ND - 1),
                                )
                            nc.vector.tensor_scalar_add(
                                dsts[0][:, sl], psq[0:HD, :], b_sb[0:HD, hp:hp + 1])
                            nc.vector.tensor_scalar_add(
                                dsts[1][:, sl], psq[HD:P, :], b_sb[HD:P, hp:hp + 1])
                        emitters.append(emit)
                return qTs, kTs, emitters

            d_done = set()

            def emit_d_chunk(qc):
                """Output projection for q-chunk qc: out = sum_f stg_f @ wo_f."""
                if qc in d_done:
                    return
                d_done.add(qc)
                for mm in range(QW // P):
                    m = qc * (QW // P) + mm
                    off = mm * P
                    for n in range(D // WD):
                        pso = s_pool(slot[0]).tile(
                            [P, WD], F32, tag=f"s{slot[0] % 2}", name="pso"
                        )
                        slot[0] += 1
                        for f in range(NM):
                            nc.tensor.matmul(
                                pso[:], stg_all[:, qc, f, off:off + P],
                                wo_sb[:, f, n * WD:(n + 1) * WD],
                                start=(f == 0), stop=(f == NM - 1),
                            )
                        ost = ostg_pool.tile([P, WD], F32, tag="ostg", name="ost")
                        nc.vector.tensor_copy(ost[:], pso[:])
                        nc.sync.dma_start(
                            out_d[m * P:(m + 1) * P, n * WD:(n + 1) * WD], ost[:]
                        )

            def norm_chunk(hp, rr, qc, o_acc):
                """Normalize finished O^T chunk: rows 0..HD-1 scaled by
                1/sums (row HD), written into stg. rr=1 writes stg rows
                0-63 directly; rr=0 goes via tmpn + partition-shift DMA."""
                srow = norm_pool.tile([1, QW], F32, tag="srow", name="srow")
                nc.vector.tensor_copy(srow[:], o_acc[HD:VW, :])
                rd1 = sums_dram.tile([1, QW], F32, tag="rd1", name="rd1")
                rd1_ap = rd1[:]
                nc.sync.dma_start(rd1_ap, srow[:])
                sq = norm_pool.tile([P, SQF], F32, tag="sq", name="sq")
                nc.sync.dma_start(
                    sq[:],
                    bass.AP(tensor=rd1_ap.tensor, offset=rd1_ap.offset,
                            ap=[[SQF, P], [1, SQF]]),
                )
                nc.vector.reciprocal(sq[:], sq[:])
                rd2 = sums_dram.tile([1, QW], F32, tag="rd2", name="rd2")
                rd2_ap = rd2[:]
                nc.sync.dma_start(
                    bass.AP(tensor=rd2_ap.tensor, offset=rd2_ap.offset,
                            ap=[[SQF, P], [1, SQF]]),
                    sq[:],
                )
                bc = norm_pool.tile([HD, QW], F32, tag="bc", name="bc")
                nc.sync.dma_start(
                    bc[:],
                    bass.AP(tensor=rd2_ap.tensor, offset=rd2_ap.offset,
                            ap=[[0, HD], [1, QW]]),
                )
                if rr == 1:
                    nc.vector.tensor_mul(
                        stg_all[0:HD, qc, hp, :], o_acc[0:HD, :], bc[:]
                    )
                else:
                    tmpn = norm_pool.tile([HD, QW], MD, tag="tmpn", name="tmpn")
                    nc.vector.tensor_mul(tmpn[:], o_acc[0:HD, :], bc[:])
                    nc.sync.dma_start(stg_all[HD:P, qc, hp, :], tmpn[:])

            # ---- attention over head pairs -------------------------------
            cur = c1_emitters(0, slot)
            for em in cur[2]:
                em()

            for hp in range(NM):
                qTs, kTs = cur[0], cur[1]
                if hp + 1 < NM:
                    nxt = c1_emitters(hp + 1, slot)
                else:
                    nxt = None
                    xT_ctx.close()

                pend_pv = [None]  # delayed-by-one-step PV emitter

                def emit_scores(rr, ch, ki):
                    W0 = ch * SW
                    qlo = max(W0, ki * P)
                    rel = qlo - W0
                    s_ps = s_pool(slot[0]).tile(
                        [P, SW], F32, tag=f"s{slot[0] % 2}", name="s_ps"
                    )
                    slot[0] += 1
                    plo = rel
                    while plo < SW:
                        pw = min(512 - plo % 512, SW - plo)
                        nc.tensor.matmul(
                            s_ps[:, plo:plo + pw],
                            kTs[rr][:, ki * P:(ki + 1) * P],
                            qTs[rr][:, W0 + plo:W0 + plo + pw],
                            start=True, stop=True,
                        )
                        plo += pw
                    pT = pT_pool.tile([P, SW], MD, tag="pT", name="pT")
                    nc.scalar.activation(
                        pT[:, rel:SW], s_ps[:, rel:SW],
                        mybir.ActivationFunctionType.Exp, scale=SCALE,
                    )
                    if ki * P >= W0:
                        # diagonal tile: zero the strictly-lower part
                        nc.vector.tensor_mul(
                            pT[:, rel:rel + P], pT[:, rel:rel + P], ut1[:]
                        )
                    return pT, rel

                def make_pv(rr, ch, ki, pT, rel, o_ps):
                    def emit_pv():
                        W0 = ch * SW
                        h = 2 * hp + rr
                        plo = rel
                        while plo < SW:
                            pw = min(512 - plo % 512, SW - plo)
                            qc = (W0 + plo) // QW
                            lastki = qc * (QW // P) + (QW // P) - 1
                            if o_ps[rr][qc] is None:
                                o_ps[rr][qc] = ps_o.tile(
                                    [VW, QW], F32, tag="o", name="o_ps"
                                )
                            nc.tensor.matmul(
                                o_ps[rr][qc][:, (W0 + plo) % QW:(W0 + plo) % QW + pw],
                                v_sb[:, ki, h, :],
                                pT[:, plo:plo + pw],
                                start=(ki == 0), stop=(ki == lastki),
                            )
                            plo += pw
                            if ki == lastki:
                                norm_chunk(hp, rr, qc, o_ps[rr][qc])
                                o_ps[rr][qc] = None
                                if hp == NM - 1 and rr == 1:
                                    emit_d_chunk(qc)
                    return emit_pv

                # fillers per window: V tiles 8-15 go into (hp0, ch0); the
                # next head-pair's QK projection spreads over both windows.
                fill = {0: [], 1: []}
                if hp == 0:
                    fill[0] = [lambda t=t: emit_v_tile(t) for t in range(NT // 2, NT)]
                    if nxt is not None:
                        fill[1] = list(nxt[2])
                elif nxt is not None:
                    fill[0] = list(nxt[2][:3])
                    fill[1] = list(nxt[2][3:])

                o_ps = [[None] * NQC for _ in range(2)]
                for ch in range(NCH):
                    nki = ((ch + 1) * SW) // P
                    steps = [(rr, ki) for ki in range(nki) for rr in (0, 1)]
                    fillers = fill[ch]
                    emitted = 0
                    for i, (rr, ki) in enumerate(steps):
                        pT, rel = emit_scores(rr, ch, ki)
                        if pend_pv[0] is not None:
                            pend_pv[0]()
                        pend_pv[0] = make_pv(rr, ch, ki, pT, rel, o_ps)
                        want = ((i + 1) * len(fillers)) // len(steps)
                        while emitted < want:
                            fillers[emitted]()
                            emitted += 1
                # flush the delayed PV at the end of the head-pair
                if pend_pv[0] is not None:
                    pend_pv[0]()
                    pend_pv[0] = None
                if hp == 0:
                    wv_ctx.close()
                cur = nxt

            # ---- phase D remainder ---------------------------------------
            for qc in range(NQC):
                emit_d_chunk(qc)

    nc.compile()
    return nc


_NC_CACHE = {}


def _get_nc(S, D, HN, HD):
    key = (S, D, HN, HD)
    if key not in _NC_CACHE:
        _NC_CACHE[key] = build_nc(S, D, HN, HD)
    return _NC_CACHE[key]


def augment_v(wv_local, bv_local, HN, HD):
    """Append per head a zero weight column with bias 1.0 (softmax-sum col)."""
    D = wv_local.shape[0]
    wv_a = np.zeros((D, HN, HD + 1), dtype=np.float32)
    wv_a[:, :, :HD] = wv_local.reshape(D, HN, HD)
    bv_a = np.ones((HN, HD + 1), dtype=np.float32)
    bv_a[:, :HD] = bv_local.reshape(HN, HD)
    return np.ascontiguousarray(wv_a.reshape(D, -1)), np.ascontiguousarray(bv_a.reshape(-1))


def kernel(**inputs):
    out, _ = run_with_results(inputs)
    return out


def run_with_results(inputs, **spmd_kwargs):
    from concourse.bass_utils import run_bass_kernel_spmd
    import ml_dtypes

    bf16 = ml_dtypes.bfloat16

    x = np.asarray(inputs["x"], dtype=np.float32)
    wq = np.asarray(inputs["wq"], dtype=np.float32)
    bq = np.asarray(inputs["bq"], dtype=np.float32)
    wk = np.asarray(inputs["wk"], dtype=np.float32)
    bk = np.asarray(inputs["bk"], dtype=np.float32)
    wv = np.asarray(inputs["wv"], dtype=np.float32)
    bv = np.asarray(inputs["bv"], dtype=np.float32)
    wo = np.asarray(inputs["wo"], dtype=np.float32)
    bo = np.asarray(inputs["bo"], dtype=np.float32)

    B, S, D = x.shape
    H = 16
    HD = D // H
    G = 2                  # head groups
    HN = H // G            # heads per core
    C = HN * HD
    n_cores = B * G

    nc = _get_nc(S, D, HN, HD)

    in_maps = []
    for c in range(n_cores):
        b, g = c // G, c % G
        sl = slice(g * C, (g + 1) * C)
        # stg packs the odd head in rows 0-63 and the even head in rows
        # 64-127 of each 128-row block; swap wo's rows to match.
        wo_loc = wo[sl, :].reshape(HN // 2, 2, HD, D)[:, ::-1]
        wo_loc = np.ascontiguousarray(wo_loc.reshape(C, D))
        wv_a, bv_a = augment_v(wv[:, sl], bv[sl], HN, HD)
        in_maps.append({
            "xT": np.ascontiguousarray(x[b].T).astype(bf16),
            "wq": np.ascontiguousarray(wq[:, sl]).astype(bf16),
            "wk": np.ascontiguousarray(wk[:, sl]).astype(bf16),
            "wv": wv_a.astype(bf16),
            "wo": wo_loc.astype(bf16),
            "bq": np.ascontiguousarray(bq[sl]),
            "bk": np.ascontiguousarray(bk[sl]),
            "bv": bv_a,
        })

    res = run_bass_kernel_spmd(nc, in_maps, core_ids=list(range(n_cores)), **spmd_kwargs)
    outs = [m["out"] for m in res.results]
    out = np.stack([sum(outs[b * G + g] for g in range(G)) for b in range(B)])
    out = out + bo[None, None, :]
    return out.astype(np.float32), res


# revision 14
# speedup vs baseline: 1.1752x; 1.0230x over previous
"""Trainium2 Bass kernel for nn_MultiHeadAttention (B=4, S=2048, D=1024, H=16).

Sharding: 8 cores = 4 batches x 2 head-groups. Core c handles batch c//2 and
heads (c%2)*8 .. (c%2)*8+7. Each core computes Q/K/V projections for its 512
columns, causal attention for its 8 heads, and a partial output projection
(row-parallel over wo). Host sums the two partials per batch and adds bo.

Per-core layout (all matmul operands bf16, fp32 PSUM accumulation):
  x^T [D, S] is shipped pre-transposed from the host. Q^T/K^T per head
  [64, S] (head dim on partitions, so scores contract over head_dim).
  V natural [S, 65] per head with a trailing ones column (augmented
  weights/bias) so the P^T@V matmul also produces softmax row-sums.
  Attention is computed transposed: S^T = K^T.T @ Q^T per 128-k-tile into
  [128, 1024] PSUM windows, exp on ACT, O^T accumulates in [65, 512] PSUM
  chunks (row 64 = sums).

Scheduling (the point of this version): the two heads of a head-pair are
interleaved step-by-step (rr alternates per ki) and the PV matmul of step s
is emitted one step late, so the PE never sits behind the exp of the score
tile it just produced. All deferrable projection work (V tiles 8-15, the
next head-pair's Q/K projection, the output projection of finished q-chunks)
is spread evenly into the attention windows as PE filler so the tensor
engine stays busy and the HAM clock gate keeps the PE at 2.4 GHz.
Normalization multiplies O^T rows by 1/sums broadcast via small DRAM
round-trips; the odd (rr=1) head lands in stg rows 0-63 by a direct DVE
write (wo rows are swapped host-side to match) since it finishes last.
"""

import numpy as np

import concourse.bass as bass
import concourse.mybir as mybir
import concourse.tile as tile
from concourse import bacc
from concourse.masks import make_upper_triangular

F32 = mybir.dt.float32
BF16 = mybir.dt.bfloat16
P = 128


def build_nc(S=2048, D=1024, HN=8, HD=64):
    """Per-core Bass module. HN = local heads, C = HN*HD local cols."""
    MD = BF16
    C = HN * HD
    NT = S // P        # token tiles
    ND = D // P        # d tiles (contraction for projections)
    NM = C // P        # head-pairs (2 heads of HD=64 per 128-partition tile)
    QW = 1024          # O^T psum chunk width (two PSUM banks; chunk == window)
    SW = 1024          # scores window width (two PSUM banks)
    W5 = 512           # projection N-chunk width
    WD = 512           # phase-D N-chunk width
    NCH = S // SW      # score windows per head
    NQC = S // QW      # O^T chunks per head
    SQF = QW // P      # free size of the [P, SQF] reciprocal reshape
    SCALE = 1.0 / float(np.sqrt(HD))
    VW = HD + 1        # V columns per head incl. trailing ones column
    CV = HN * VW       # augmented V cols

    nc = bacc.Bacc("TRN2", target_bir_lowering=False)

    xT_d = nc.dram_tensor("xT", [D, S], MD, kind="ExternalInput")
    wq_d = nc.dram_tensor("wq", [D, C], MD, kind="ExternalInput")
    wk_d = nc.dram_tensor("wk", [D, C], MD, kind="ExternalInput")
    wv_d = nc.dram_tensor("wv", [D, CV], MD, kind="ExternalInput")
    wo_d = nc.dram_tensor("wo", [C, D], MD, kind="ExternalInput")
    bq_d = nc.dram_tensor("bq", [C], F32, kind="ExternalInput")
    bk_d = nc.dram_tensor("bk", [C], F32, kind="ExternalInput")
    bv_d = nc.dram_tensor("bv", [CV], F32, kind="ExternalInput")
    out_d = nc.dram_tensor("out", [S, D], F32, kind="ExternalOutput")

    with tile.TileContext(nc) as tc:
        from contextlib import ExitStack

        with ExitStack() as ctx:
            singles = ctx.enter_context(tc.tile_pool(name="singles", bufs=1))
            # ut1[k, q] = 1.0 where k <= q else 0 (valid causal region of a
            # diagonal tile in S^T = [k, q] layout).
            ut1 = singles.tile([P, P], MD)
            make_upper_triangular(nc, ut1[:], val=1.0, diag=True)

            bq_sb = singles.tile([P, NM], F32)
            nc.sync.dma_start(bq_sb[:], bq_d.rearrange("(m p) -> p m", p=P))
            bk_sb = singles.tile([P, NM], F32)
            nc.sync.dma_start(bk_sb[:], bk_d.rearrange("(m p) -> p m", p=P))
            # bv broadcast to all partitions via step-0 partition DMA read.
            bv_sb = singles.tile([P, CV], F32)
            nc.sync.dma_start(
                bv_sb[:], bass.AP(tensor=bv_d, offset=0, ap=[[0, P], [1, CV]])
            )

            wqk_pool = ctx.enter_context(tc.tile_pool(name="wqk", bufs=1))
            wq_sb = wqk_pool.tile([P, ND, C], MD)
            wk_sb = wqk_pool.tile([P, ND, C], MD)

            # PSUM: two score slots (2 banks each) + 4 O^T slots (1 bank
            # each) = 8 banks. Projection psums tag-share the score slots.
            ps_s0 = ctx.enter_context(tc.tile_pool(name="ps_s0", bufs=1, space="PSUM"))
            ps_s1 = ctx.enter_context(tc.tile_pool(name="ps_s1", bufs=1, space="PSUM"))
            ps_o = ctx.enter_context(tc.tile_pool(name="ps_o", bufs=2, space="PSUM"))

            def s_pool(i):
                return ps_s0 if i % 2 == 0 else ps_s1

            # Mid-kernel-released pools on the right-side stack: wv first,
            # xT last.
            xT_ctx = ExitStack()
            xT_pool = xT_ctx.enter_context(tc.tile_pool(name="xT", bufs=1, side="right"))
            wv_ctx = ExitStack()
            wv_pool = wv_ctx.enter_context(tc.tile_pool(name="wv", bufs=1, side="right"))

            xT = xT_pool.tile([P, ND, S], MD)
            wv_sb = wv_pool.tile([P, ND, CV], MD)
            nc.sync.dma_start(wv_sb[:], wv_d.rearrange("(o p) n -> p o n", p=P))
            # x^T arrives in token-quarters so V tiles can start early.
            xT_src = xT_d.rearrange("(o p) n -> p o n", p=P)
            TQ = S // 4
            for i in range(4):
                nc.sync.dma_start(
                    xT[:, :, i * TQ:(i + 1) * TQ], xT_src[:, :, i * TQ:(i + 1) * TQ]
                )

            v_pool = ctx.enter_context(tc.tile_pool(name="v", bufs=1))
            v_sb = v_pool.tile([P, NT, HN, VW], MD)
            v_pieces = [(lo, min(512, CV - lo)) for lo in range(0, CV, 512)]

            slot = [0]   # shared psum score-slot parity counter

            def emit_v_tile(t):
                psv = s_pool(slot[0]).tile([P, CV], F32, tag=f"s{slot[0] % 2}", name="psv")
                slot[0] += 1
                for d in range(ND):
                    for lo, w in v_pieces:
                        nc.tensor.matmul(
                            psv[:, lo:lo + w], xT[:, d, t * P:(t + 1) * P],
                            wv_sb[:, d, lo:lo + w],
                            start=(d == 0), stop=(d == ND - 1),
                        )
                nc.vector.tensor_add(v_sb[:, t], psv[:], bv_sb[:])

            # V projection for the first NT//2 token tiles runs up front;
            # the rest interleaves into head-pair 0's first window.
            for t in range(NT // 2):
                emit_v_tile(t)

            # Q/K weights after x/wv so the V pipeline starts sooner.
            nc.sync.dma_start(wq_sb[:], wq_d.rearrange("(o p) n -> p o n", p=P))
            nc.sync.dma_start(wk_sb[:], wk_d.rearrange("(o p) n -> p o n", p=P))

            wo_pool = ctx.enter_context(tc.tile_pool(name="wo", bufs=1))
            wo_sb = wo_pool.tile([P, NM, D], MD)
            nc.sync.dma_start(wo_sb[:], wo_d.rearrange("(f p) n -> p f n", p=P))

            # normalized O^T tiles until phase D; rows 0-63 = odd head (rr=1,
            # DVE-direct), rows 64-127 = even head (rr=0, via DMA). wo rows
            # are swapped host-side to match.
            stg_pool = ctx.enter_context(tc.tile_pool(name="stg", bufs=1))
            stg_all = stg_pool.tile([P, NQC, NM, QW], MD)
            qkT_pool = ctx.enter_context(tc.tile_pool(name="qkT", bufs=2))
            pT_pool = ctx.enter_context(tc.tile_pool(name="pT", bufs=4))
            norm_pool = ctx.enter_context(tc.tile_pool(name="norm", bufs=2))
            sums_dram = ctx.enter_context(tc.tile_pool(name="sumsd", bufs=4, space="DRAM"))
            ostg_pool = ctx.enter_context(tc.tile_pool(name="ostg", bufs=3))

            def c1_emitters(hp, slot):
                """Q^T/K^T projection for head-pair hp: qTs/kTs tiles plus a
                list of 8 emitter closures (one per psum accumulation group)
                for fine-grained interleaved emission."""
                qT0 = qkT_pool.tile([HD, S], MD, tag="qT0", name="qT0")
                qT1 = qkT_pool.tile([HD, S], MD, tag="qT1", name="qT1")
                kT0 = qkT_pool.tile([HD, S], MD, tag="kT0", name="kT0")
                kT1 = qkT_pool.tile([HD, S], MD, tag="kT1", name="kT1")
                qTs = [qT0[:], qT1[:]]
                kTs = [kT0[:], kT1[:]]
                emitters = []
                for n in range(S // W5):
                    for w_sb, b_sb, dsts in ((wq_sb, bq_sb, qTs), (wk_sb, bk_sb, kTs)):
                        def emit(n=n, w_sb=w_sb, b_sb=b_sb, dsts=dsts):
                            sl = slice(n * W5, (n + 1) * W5)
                            psq = s_pool(slot[0]).tile(
                                [P, W5], F32, tag=f"s{slot[0] % 2}", name="psq"
                            )
                            slot[0] += 1
                            for d in range(ND):
                                nc.tensor.matmul(
                                    psq[:], w_sb[:, d, hp * P:(hp + 1) * P],
                                    xT[:, d, sl],
                                    start=(d == 0), stop=(d == ND - 1),
                                )
                            nc.vector.tensor_scalar_add(
                                dsts[0][:, sl], psq[0:HD, :], b_sb[0:HD, hp:hp + 1])
                            nc.vector.tensor_scalar_add(
                                dsts[1][:, sl], psq[HD:P, :], b_sb[HD:P, hp:hp + 1])
                        emitters.append(emit)
                return qTs, kTs, emitters

            d_done = set()
            d_fill = []   # phase-D pso closures, pumped as PE filler

            def queue_d_chunk(qc):
                """Output projection for q-chunk qc: out = sum_f stg_f @ wo_f.
                Queued as 16 independent closures so the hp3 step loop can
                spread them as PE filler."""
                if qc in d_done:
                    return
                d_done.add(qc)
                for mm in range(QW // P):
                    m = qc * (QW // P) + mm
                    off = mm * P
                    for n in range(D // WD):
                        def go(m=m, off=off, n=n, qc=qc):
                            pso = s_pool(slot[0]).tile(
                                [P, WD], F32, tag=f"s{slot[0] % 2}", name="pso"
                            )
                            slot[0] += 1
                            for f in range(NM):
                                nc.tensor.matmul(
                                    pso[:], stg_all[:, qc, f, off:off + P],
                                    wo_sb[:, f, n * WD:(n + 1) * WD],
                                    start=(f == 0), stop=(f == NM - 1),
                                )
                            ost = ostg_pool.tile([P, WD], F32, tag="ostg", name="ost")
                            nc.vector.tensor_copy(ost[:], pso[:])
                            nc.sync.dma_start(
                                out_d[m * P:(m + 1) * P, n * WD:(n + 1) * WD], ost[:]
                            )
                        d_fill.append(go)

            def norm_chunk(hp, rr, qc, o_acc):
                """Normalize finished O^T chunk: rows 0..HD-1 scaled by
                1/sums (row HD), written into stg. O^T and the sums row are
                evacuated to SBUF right away (frees the PSUM slot); the
                multiply runs on the otherwise-idle GpSimd engine. rr=1
                writes stg rows 0-63 directly; rr=0 goes via tmpn + a
                partition-shift DMA."""
                oc = norm_pool.tile([HD, QW], F32, tag="oc", name="oc")
                nc.vector.tensor_copy(oc[:], o_acc[0:HD, :])
                srow = norm_pool.tile([1, QW], F32, tag="srow", name="srow")
                nc.vector.tensor_copy(srow[:], o_acc[HD:VW, :])
                rd1 = sums_dram.tile([1, QW], F32, tag="rd1", name="rd1")
                rd1_ap = rd1[:]
                nc.sync.dma_start(rd1_ap, srow[:])
                sq = norm_pool.tile([P, SQF], F32, tag="sq", name="sq")
                nc.sync.dma_start(
                    sq[:],
                    bass.AP(tensor=rd1_ap.tensor, offset=rd1_ap.offset,
                            ap=[[SQF, P], [1, SQF]]),
                )
                nc.vector.reciprocal(sq[:], sq[:])
                rd2 = sums_dram.tile([1, QW], F32, tag="rd2", name="rd2")
                rd2_ap = rd2[:]
                nc.sync.dma_start(
                    bass.AP(tensor=rd2_ap.tensor, offset=rd2_ap.offset,
                            ap=[[SQF, P], [1, SQF]]),
                    sq[:],
                )
                bc = norm_pool.tile([HD, QW], F32, tag="bc", name="bc")
                nc.sync.dma_start(
                    bc[:],
                    bass.AP(tensor=rd2_ap.tensor, offset=rd2_ap.offset,
                            ap=[[0, HD], [1, QW]]),
                )
                if rr == 1:
                    nc.gpsimd.tensor_mul(
                        stg_all[0:HD, qc, hp, :], oc[:], bc[:]
                    )
                else:
                    tmpn = norm_pool.tile([HD, QW], MD, tag="tmpn", name="tmpn")
                    nc.gpsimd.tensor_mul(tmpn[:], oc[:], bc[:])
                    nc.sync.dma_start(stg_all[HD:P, qc, hp, :], tmpn[:])

            # ---- attention over head pairs -------------------------------
            cur = c1_emitters(0, slot)
            for em in cur[2]:
                em()

            for hp in range(NM):
                qTs, kTs = cur[0], cur[1]
                if hp + 1 < NM:
                    nxt = c1_emitters(hp + 1, slot)
                else:
                    nxt = None
                    xT_ctx.close()

                pend_pv = [None]  # delayed-by-one-step PV emitter

                def emit_scores(rr, ch, ki):
                    W0 = ch * SW
                    qlo = max(W0, ki * P)
                    rel = qlo - W0
                    s_ps = s_pool(slot[0]).tile(
                        [P, SW], F32, tag=f"s{slot[0] % 2}", name="s_ps"
                    )
                    slot[0] += 1
                    plo = rel
                    while plo < SW:
                        pw = min(512 - plo % 512, SW - plo)
                        nc.tensor.matmul(
                            s_ps[:, plo:plo + pw],
                            kTs[rr][:, ki * P:(ki + 1) * P],
                            qTs[rr][:, W0 + plo:W0 + plo + pw],
                            start=True, stop=True,
                        )
                        plo += pw
                    # keep-alive: a tiny LDWEIGHTS each step keeps the PE HAM
                    # activity monitor from re-throttling the clock during
                    # ACT-paced stretches.
                    nc.tensor.ldweights(ut1[:, 0:16])
                    pT = pT_pool.tile([P, SW], MD, tag="pT", name="pT")
                    nc.scalar.activation(
                        pT[:, rel:SW], s_ps[:, rel:SW],
                        mybir.ActivationFunctionType.Exp, scale=SCALE,
                    )
                    if ki * P >= W0:
                        # diagonal tile: zero the strictly-lower part (on the
                        # otherwise-idle GpSimd engine, off the DVE queue)
                        nc.gpsimd.tensor_mul(
                            pT[:, rel:rel + P], pT[:, rel:rel + P], ut1[:]
                        )
                    return pT, rel

                def make_pv(rr, ch, ki, pT, rel, o_ps):
                    def emit_pv():
                        W0 = ch * SW
                        h = 2 * hp + rr
                        plo = rel
                        done = False
                        while plo < SW:
                            pw = min(512 - plo % 512, SW - plo)
                            qc = ch
                            lastki = (ch + 1) * (SW // P) - 1
                            if o_ps[rr][qc] is None:
                                o_ps[rr][qc] = ps_o.tile(
                                    [VW, QW], F32, tag="o", name="o_ps"
                                )
                            nc.tensor.matmul(
                                o_ps[rr][qc][:, plo:plo + pw],
                                v_sb[:, ki, h, :],
                                pT[:, plo:plo + pw],
                                start=(ki == 0), stop=(ki == lastki),
                            )
                            plo += pw
                            done = ki == lastki
                        if done:
                            qc = ch
                            norm_chunk(hp, rr, qc, o_ps[rr][qc])
                            o_ps[rr][qc] = None
                            if hp == NM - 1 and rr == 1:
                                queue_d_chunk(qc)
                    return emit_pv

                # fillers per window: V tiles 8-15 go into (hp0, ch0); the
                # next head-pair's QK projection spreads over both windows.
                fill = {0: [], 1: []}
                if hp == 0:
                    fill[0] = [lambda t=t: emit_v_tile(t) for t in range(NT // 2, NT)]
                    if nxt is not None:
                        fill[1] = list(nxt[2])
                elif nxt is not None:
                    fill[0] = list(nxt[2][:3])
                    fill[1] = list(nxt[2][3:])

                o_ps = [[None] * NQC for _ in range(2)]
                for ch in range(NCH):
                    nki = ((ch + 1) * SW) // P
                    steps = [(rr, ki) for ki in range(nki) for rr in (0, 1)]
                    fillers = fill[ch]
                    emitted = 0
                    for i, (rr, ki) in enumerate(steps):
                        pT, rel = emit_scores(rr, ch, ki)
                        if pend_pv[0] is not None:
                            pend_pv[0]()
                        pend_pv[0] = make_pv(rr, ch, ki, pT, rel, o_ps)
                        want = ((i + 1) * len(fillers)) // len(steps)
                        while emitted < want:
                            fillers[emitted]()
                            emitted += 1
                        if d_fill:
                            d_fill.pop(0)()
                # flush the delayed PV at the end of the head-pair
                if pend_pv[0] is not None:
                    pend_pv[0]()
                    pend_pv[0] = None
                if hp == 0:
                    wv_ctx.close()
                cur = nxt

            # ---- phase D remainder ---------------------------------------
            for qc in range(NQC):
                queue_d_chunk(qc)
            while d_fill:
                d_fill.pop(0)()

    nc.compile()
    return nc


_NC_CACHE = {}


def _get_nc(S, D, HN, HD):
    key = (S, D, HN, HD)
    if key not in _NC_CACHE:
        _NC_CACHE[key] = build_nc(S, D, HN, HD)
    return _NC_CACHE[key]


def augment_v(wv_local, bv_local, HN, HD):
    """Append per head a zero weight column with bias 1.0 (softmax-sum col)."""
    D = wv_local.shape[0]
    wv_a = np.zeros((D, HN, HD + 1), dtype=np.float32)
    wv_a[:, :, :HD] = wv_local.reshape(D, HN, HD)
    bv_a = np.ones((HN, HD + 1), dtype=np.float32)
    bv_a[:, :HD] = bv_local.reshape(HN, HD)
    return np.ascontiguousarray(wv_a.reshape(D, -1)), np.ascontiguousarray(bv_a.reshape(-1))


def kernel(**inputs):
    out, _ = run_with_results(inputs)
    return out


def run_with_results(inputs, **spmd_kwargs):
    from concourse.bass_utils import run_bass_kernel_spmd
    import ml_dtypes

    bf16 = ml_dtypes.bfloat16

    x = np.asarray(inputs["x"], dtype=np.float32)
    wq = np.asarray(inputs["wq"], dtype=np.float32)
    bq = np.asarray(inputs["bq"], dtype=np.float32)
    wk = np.asarray(inputs["wk"], dtype=np.float32)
    bk = np.asarray(inputs["bk"], dtype=np.float32)
    wv = np.asarray(inputs["wv"], dtype=np.float32)
    bv = np.asarray(inputs["bv"], dtype=np.float32)
    wo = np.asarray(inputs["wo"], dtype=np.float32)
    bo = np.asarray(inputs["bo"], dtype=np.float32)

    B, S, D = x.shape
    H = 16
    HD = D // H
    G = 2                  # head groups
    HN = H // G            # heads per core
    C = HN * HD
    n_cores = B * G

    nc = _get_nc(S, D, HN, HD)

    in_maps = []
    for c in range(n_cores):
        b, g = c // G, c % G
        sl = slice(g * C, (g + 1) * C)
        # stg packs the odd head in rows 0-63 and the even head in rows
        # 64-127 of each 128-row block; swap wo's rows to match.
        wo_loc = wo[sl, :].reshape(HN // 2, 2, HD, D)[:, ::-1]
        wo_loc = np.ascontiguousarray(wo_loc.reshape(C, D))
        wv_a, bv_a = augment_v(wv[:, sl], bv[sl], HN, HD)
        in_maps.append({
            "xT": np.ascontiguousarray(x[b].T).astype(bf16),
            "wq": np.ascontiguousarray(wq[:, sl]).astype(bf16),
            "wk": np.ascontiguousarray(wk[:, sl]).astype(bf16),
            "wv": wv_a.astype(bf16),
            "wo": wo_loc.astype(bf16),
            "bq": np.ascontiguousarray(bq[sl]),
            "bk": np.ascontiguousarray(bk[sl]),
            "bv": bv_a,
        })

    res = run_bass_kernel_spmd(nc, in_maps, core_ids=list(range(n_cores)), **spmd_kwargs)
    outs = [m["out"] for m in res.results]
    out = np.stack([sum(outs[b * G + g] for g in range(G)) for b in range(B)])
    out = out + bo[None, None, :]
    return out.astype(np.float32), res
